# revision 7
# baseline (speedup 1.0000x reference)
"""GCN layer (PyG GCNConv semantics) on 8 Trainium2 NeuronCores.

out = D^{-1/2} (A + I) D^{-1/2} (x @ W) + b

Strategy (graph/data parallel, destinations sharded across cores):
  - Factor the symmetric norm: out = dinv * ((A+I) @ (dinv * (x@W))) + b.
  - Every core computes the full h' = dinv * (x @ W) with TensorE
    (x^T is host-transposed/bf16-cast; dinv = rsqrt(deg) on-device).
  - h' is split into an fp8e4 pair (h8 = fp8(h'), r8 = fp8(h' - h8));
    the sum h8+r8 carries ~8 mantissa bits, matching bf16 accuracy.
  - Each core owns a 1250-destination slice. The host re-encodes its
    edge bucket as a dense count matrix A_c [10240 src, 1250 dst]
    (fp8e4: counts <=16 are exact; self-loops included) —
    a pure structural re-encoding, streamed tile-by-tile at line rate.
  - TensorE contracts with fp8 DoubleRow matmuls (2 source tiles per
    instruction at 0.5 cycles/output-column — 4 A-columns/cycle):
    acc^T[f, dst] += h8_pair^T @ A_pair + r8_pair^T @ A_pair,
    accumulating all 40 pairs x 2 passes in PSUM. This halves TensorE
    time vs the bf16xfp8 stream while keeping bf16-level accuracy.
  - Postscale by dinv[dst], add bias, write out^T; host reassembles.
"""

import sys

for _p in ("/opt/trn_rl_repo", "/root/.axon_site/_ro/trn_rl_repo"):
    if _p not in sys.path:
        sys.path.append(_p)

import numpy as np
import ml_dtypes

N_NODES = 10000
N_CORES = 8
PER_CORE = 1250  # dst nodes per core
D = 128
NPAD = 10240  # padded node count (80 tiles of 128)
NTILE = NPAD // 128  # 80
DSTPAD = 1250  # per-core dst count
NTB = 80  # source tiles streamed in phase B (tile 79 is all padding)
NPAIR = NTB // 2  # 40 DoubleRow tile pairs
APAD = NTB * 128  # 10240 rows of A
TPD = 8  # A tiles per DMA group (4 pairs)
NG = NTB // TPD  # 10 DMA groups
# phase-B dst chunks: DoubleRow moving free dim is 2*chunk <= 512
CHUNKS = [(0, 256), (256, 256), (512, 256), (768, 256), (1024, 226)]
LAG = 2  # pairs of pipeline slack between quantization and phase B

_cache = {}


def _build_program(reps=1, a_dtype="float8e4"):
    """Build + finalize the SPMD Bass program (shape-independent).

    reps > 1 wraps the computation in a device-side For_i loop (for timing:
    the axon RPC wall-clock floor is ~100ms, so K iterations on-device make
    the kernel time measurable as a slope)."""
    import concourse.bacc as bacc
    import concourse.mybir as mybir
    import concourse.tile as tile

    nc = bacc.Bacc(None)
    bf16 = mybir.dt.bfloat16
    f32 = mybir.dt.float32
    f8 = mybir.dt.float8e4
    adt = getattr(mybir.dt, a_dtype)
    fp8_path = a_dtype == "float8e4"

    xT_p = nc.declare_dram_parameter("xT", [128, NPAD], bf16, isOutput=False)
    w_p = nc.declare_dram_parameter("W", [128, 128], bf16, isOutput=False)
    deg2d_p = nc.declare_dram_parameter("deg2d", [128, NTILE], f32, isOutput=False)
    degw_p = nc.declare_dram_parameter("degw", [128, DSTPAD], f32, isOutput=False)
    bias_p = nc.declare_dram_parameter("bias", [128, 1], f32, isOutput=False)
    a_p = nc.declare_dram_parameter("A", [APAD, DSTPAD], adt, isOutput=False)
    out_p = nc.declare_dram_parameter("out", [128, DSTPAD], f32, isOutput=True)

    with tile.TileContext(nc) as tc:
        with (
            tc.tile_pool(name="persist", bufs=1) as pp,
            tc.tile_pool(name="hps", bufs=2, space="PSUM") as hps,
            tc.tile_pool(name="aps", bufs=1, space="PSUM") as aps,
            tc.tile_pool(name="ap_sb", bufs=4) as ap_sb,
        ):
            xT = pp.tile([128, NPAD], bf16)
            XCH = 4  # load x^T in 4 slices so matmuls can start early
            for i in range(XCH):
                sl = slice(i * NPAD // XCH, (i + 1) * NPAD // XCH)
                nc.sync.dma_start(xT[:, sl], xT_p[:, sl])
            w_sb = pp.tile([128, 128], bf16)
            nc.sync.dma_start(w_sb[:], w_p[:])
            deg2d = pp.tile([128, NTILE], f32)
            nc.sync.dma_start(deg2d[:], deg2d_p[:])
            degw = pp.tile([128, DSTPAD], f32)
            nc.sync.dma_start(degw[:], degw_p[:])
            bias_sb = pp.tile([128, 1], f32)
            nc.sync.dma_start(bias_sb[:], bias_p[:])

            # dinv = 1/sqrt(deg): reciprocal on DVE, sqrt on ACT
            # (the Rsqrt activation is banned for accuracy reasons).
            dinv2d = pp.tile([128, NTILE], f32)
            nc.vector.reciprocal(dinv2d[:], deg2d[:])
            nc.scalar.sqrt(dinv2d[:], dinv2d[:])
            dinvw = pp.tile([128, DSTPAD], f32)
            nc.vector.reciprocal(dinvw[:], degw[:])
            nc.scalar.sqrt(dinvw[:], dinvw[:])

            outsb = pp.tile([128, DSTPAD], f32)
            if fp8_path:
                h8 = pp.tile([128, NPAIR, 2, 128], f8)
                r8 = pp.tile([128, NPAIR, 2, 128], f8)
                # tile 79 is pure padding: zero once so phase B never
                # multiplies uninitialized fp8 (NaN) into PSUM
                nc.vector.memset(h8[:, NPAIR - 1, 1, :], 0.0)
                nc.vector.memset(r8[:, NPAIR - 1, 1, :], 0.0)
                state = (h8, r8)
            else:
                hsb = pp.tile([128, NPAD], bf16)
                state = (hsb,)

            emit = _emit_body_fp8 if fp8_path else _emit_body_bf16
            if reps == 1:
                emit(nc, mybir, adt, xT, w_sb, dinv2d, dinvw, bias_sb,
                     state, outsb, a_p, out_p, hps, aps, ap_sb)
            else:
                # hint_engines arms the branch prefetcher so the back-edge
                # IRAM refetch (~4us for >256-inst bodies) doesn't pollute
                # the per-iteration timing measurement
                hints = (mybir.EngineType.PE, mybir.EngineType.SP,
                         mybir.EngineType.DVE, mybir.EngineType.Activation)
                with tc.For_i(0, reps, 1, hint_engines=hints):
                    emit(nc, mybir, adt, xT, w_sb, dinv2d, dinvw, bias_sb,
                         state, outsb, a_p, out_p, hps, aps, ap_sb)

    nc.finalize()
    return nc


def _emit_body_fp8(nc, mybir, adt, xT, w_sb, dinv2d, dinvw, bias_sb, state,
                   outsb, a_p, out_p, hps, aps, ap_sb):
    h8, r8 = state
    f32 = mybir.dt.float32
    DR = mybir.MatmulPerfMode.DoubleRow
    pa = []
    for c, (w0, w) in enumerate(CHUNKS):
        pac = aps.tile([128, w], f32, tag=f"pa{c}")
        pa.append(pac)
    at_tiles = {}

    def load_group(g):
        at = ap_sb.tile([128, TPD, DSTPAD], adt, tag="at")
        nc.sync.dma_start(
            at[:],
            a_p[g * TPD * 128 : (g + 1) * TPD * 128, :].rearrange(
                "(g p) d -> p g d", p=128
            ),
        )
        at_tiles[g] = at

    def quant_tile(t, ph4):
        # h' tile in PSUM (four tiles share one full PSUM bank), then split
        # into fp8 value + fp8 residual:
        # h8 = fp8(ph*dinv) on ACT; r8 = fp8(ph*dinv - h8) fused on DVE.
        s = t % 4
        ph = ph4[:, s * 128 : (s + 1) * 128]
        nc.tensor.matmul(
            out=ph,
            lhsT=xT[:, t * 128 : (t + 1) * 128],
            rhs=w_sb[:],
            start=True,
            stop=True,
        )
        p, i = t // 2, t % 2
        nc.scalar.activation(
            h8[:, p, i, :],
            ph,
            mybir.ActivationFunctionType.Copy,
            scale=dinv2d[:, t : t + 1],
        )
        nc.vector.scalar_tensor_tensor(
            r8[:, p, i, :],
            ph,
            dinv2d[:, t : t + 1],
            h8[:, p, i, :],
            op0=mybir.AluOpType.mult,
            op1=mybir.AluOpType.subtract,
        )

    def phase_b_pair(p):
        g, gi = p // 4, (p % 4) * 2
        at = at_tiles[g]
        for c, (w0, w) in enumerate(CHUNKS):
            for k, hh in enumerate((h8, r8)):
                nc.tensor.matmul(
                    out=pa[c][:],
                    lhsT=hh[:, p, :, :],
                    rhs=at[:, gi : gi + 2, w0 : w0 + w],
                    start=(p == 0 and k == 0),
                    stop=(p == NPAIR - 1 and k == 1),
                    perf_mode=DR,
                )

    load_group(0)
    load_group(1)
    ph4 = None
    for p in range(NPAIR):
        if p % 2 == 0:
            ph4 = hps.tile([128, 512], f32, tag="ph4")
        for i in (0, 1):
            t = 2 * p + i
            if t < NTB - 1:
                quant_tile(t, ph4)
        if p % 4 == 0 and p // 4 + 2 < NG:
            load_group(p // 4 + 2)
        if p >= LAG:
            phase_b_pair(p - LAG)
    for p in range(NPAIR - LAG, NPAIR):
        phase_b_pair(p)

    # ---- postscale + bias + store ---------------------------------
    for c, (w0, w) in enumerate(CHUNKS):
        nc.vector.tensor_tensor(
            out=outsb[:, w0 : w0 + w],
            in0=pa[c][:],
            in1=dinvw[:, w0 : w0 + w],
            op=mybir.AluOpType.mult,
        )
    # bias add on ACT (per-partition bias AP); DVE stays off the critical path
    nc.scalar.add(outsb[:], outsb[:], bias_sb[:, 0:1])
    nc.sync.dma_start(out_p[:], outsb[:])


def _emit_body_bf16(nc, mybir, adt, xT, w_sb, dinv2d, dinvw, bias_sb, state,
                    outsb, a_p, out_p, hps, aps, ap_sb):
    """Fallback for pathological inputs (edge multiplicity > 15): the
    baseline bf16-A stream (1 cycle/column, no DoubleRow)."""
    (hsb,) = state
    f32 = mybir.dt.float32
    PCH = 512
    NCH = (DSTPAD + PCH - 1) // PCH
    for t in range(NTILE):
        ph = hps.tile([128, 128], f32, tag="ph")
        nc.tensor.matmul(
            out=ph[:],
            lhsT=xT[:, t * 128 : (t + 1) * 128],
            rhs=w_sb[:],
            start=True,
            stop=True,
        )
        if t % 2 == 0:
            nc.scalar.activation(
                hsb[:, t * 128 : (t + 1) * 128],
                ph[:],
                mybir.ActivationFunctionType.Copy,
                scale=dinv2d[:, t : t + 1],
            )
        else:
            nc.vector.tensor_scalar_mul(
                hsb[:, t * 128 : (t + 1) * 128], ph[:], dinv2d[:, t : t + 1]
            )

    pa = []
    for c in range(NCH):
        pac = aps.tile([128, min(PCH, DSTPAD - c * PCH)], f32, tag=f"pa{c}")
        pa.append(pac)
    for g in range(NG):
        at = ap_sb.tile([128, TPD, DSTPAD], adt, tag="at")
        nc.sync.dma_start(
            at[:],
            a_p[g * TPD * 128 : (g + 1) * TPD * 128, :].rearrange(
                "(g p) d -> p g d", p=128
            ),
        )
        for gg in range(TPD):
            t = g * TPD + gg
            for c in range(NCH):
                w0 = c * PCH
                w1 = min(w0 + PCH, DSTPAD)
                nc.tensor.matmul(
                    out=pa[c][:],
                    lhsT=hsb[:, t * 128 : (t + 1) * 128],
                    rhs=at[:, gg, w0:w1],
                    start=(t == 0),
                    stop=(t == NTB - 1),
                )
    for c in range(NCH):
        w0 = c * PCH
        w1 = min(w0 + PCH, DSTPAD)
        nc.vector.tensor_tensor(
            out=outsb[:, w0:w1],
            in0=pa[c][:],
            in1=dinvw[:, w0:w1],
            op=mybir.AluOpType.mult,
        )
    nc.vector.tensor_scalar_add(outsb[:], outsb[:], bias_sb[:, 0:1])
    nc.sync.dma_start(out_p[:], outsb[:])


def _prep_inputs(x, adj, W, b, a_dtype="float8e4"):
    """Host-side sharding/layout: per-core dense count matrix, casts,
    transposes. No numeric computation happens here (degrees are counts;
    rsqrt/scaling/matmul run on-device)."""
    bf = ml_dtypes.bfloat16
    src = np.asarray(adj[0], dtype=np.int64)
    dst = np.asarray(adj[1], dtype=np.int64)
    x = np.asarray(x, dtype=np.float32)
    W = np.asarray(W, dtype=np.float32)
    b = np.asarray(b, dtype=np.float32)
    n = x.shape[0]
    assert n == N_NODES and x.shape[1] == D

    # self-loops as ordinary edges
    loops = np.arange(n, dtype=np.int64)
    allsrc = np.concatenate([src, loops])
    alldst = np.concatenate([dst, loops])

    deg = np.bincount(alldst, minlength=n).astype(np.float32)  # includes loops
    deg_pad = np.ones(NPAD, dtype=np.float32)
    deg_pad[:n] = deg

    xpad = np.zeros((NPAD, D), dtype=np.float32)
    xpad[:n] = x
    xT = np.ascontiguousarray(xpad.T).astype(bf)
    W16 = W.astype(bf)
    deg2d = np.ascontiguousarray(deg_pad.reshape(NTILE, 128).T)
    bias = np.ascontiguousarray(b.reshape(D, 1))

    corea = alldst // PER_CORE
    loc = alldst - corea * PER_CORE
    in_maps = []
    for c in range(N_CORES):
        m = corea == c
        key = allsrc[m] * DSTPAD + loc[m]
        counts = np.bincount(key, minlength=APAD * DSTPAD)
        adt = np.dtype("float8_e4m3") if a_dtype == "float8e4" else bf
        A = counts.reshape(APAD, DSTPAD).astype(adt)
        degw = np.tile(deg_pad[c * PER_CORE : c * PER_CORE + DSTPAD][None, :], (128, 1))
        in_maps.append(
            {
                "xT": xT,
                "W": W16,
                "deg2d": deg2d,
                "degw": np.ascontiguousarray(degw),
                "bias": bias,
                "A": A,
            }
        )
    return in_maps


def kernel(x, adj, W, b):
    from concourse.bass_utils import run_bass_kernel_spmd

    # edge multiplicities up to 16 are exact in fp8e4; else use bf16
    dst = np.asarray(adj[1], dtype=np.int64)
    src = np.asarray(adj[0], dtype=np.int64)
    maxmult = int(np.bincount(src * np.int64(N_NODES) + dst).max())
    a_dtype = "float8e4" if maxmult + 1 <= 16 else "bfloat16"
    if a_dtype not in _cache:
        _cache[a_dtype] = _build_program(a_dtype=a_dtype)
    nc = _cache[a_dtype]
    in_maps = _prep_inputs(x, adj, W, b, a_dtype)
    res = run_bass_kernel_spmd(nc, in_maps, list(range(N_CORES)))
    out = np.empty((N_NODES, D), dtype=np.float32)
    for c in range(N_CORES):
        ot = res.results[c]["out"]  # [128, 1250] = out^T
        out[c * PER_CORE : (c + 1) * PER_CORE] = ot.T[:PER_CORE]
    return out


# revision 13
# speedup vs baseline: 1.4677x; 1.4677x over previous
"""GCN layer (PyG GCNConv semantics) on 8 Trainium2 NeuronCores.

out = D^{-1/2} (A + I) D^{-1/2} (x @ W) + b

Strategy (graph/data parallel, destinations sharded across cores):
  - Factor the symmetric norm and REASSOCIATE the contraction:
        out^T = W^T @ ((dinv * x)^T @ (A+I)) * dinv_dst + b
    i.e. contract the scaled features with the adjacency FIRST
    (xa[f_in, dst] = sum_s dinv_s x[s, f_in] A[s, dst]), then apply the
    128x128 weight once at the end. This removes the per-tile x@W
    matmuls entirely: TensorE does one ld+stream pass over A per source
    tile and a single trailing W application.
  - Each core owns a 1250-destination slice. The host re-encodes its
    edge bucket as a dense count matrix A_c [10112 src, 1250 dst]
    (fp8e4: counts <=16 are exact; self-loops included) —
    a pure structural re-encoding, streamed tile-by-tile at line rate.
  - x is uploaded host-swizzled into [node_lo, tile*f] tile layout; each
    iteration scales tile t by dinv (per-partition scalar on ACT/DVE)
    and TensorE contracts xd_t^T @ A_t into PSUM over all 79 tiles.
  - Tail: xa -> SBUF (bf16), out^T = W^T @ xa, postscale by dinv[dst],
    add bias, store out^T; host reassembles.
"""

import sys

for _p in ("/opt/trn_rl_repo", "/root/.axon_site/_ro/trn_rl_repo"):
    if _p not in sys.path:
        sys.path.append(_p)

import numpy as np
import ml_dtypes

N_NODES = 10000
N_CORES = 8
PER_CORE = 1250  # dst nodes per core
D = 128
NTB = 79  # source tiles (node padding to 10112; rest of x is zero)
NPAD = NTB * 128  # 10112
APAD = NPAD
DSTPAD = 1250  # per-core dst count
TPD = 8  # A tiles per DMA group
NG = (NTB + TPD - 1) // TPD  # 10 groups (last has 7)
PCH = 512  # psum chunk (max matmul moving free dim)
NCH = (DSTPAD + PCH - 1) // PCH  # 3 chunks: 512, 512, 226

_cache = {}


def _build_program(reps=1, a_dtype="float8e4"):
    """Build + finalize the SPMD Bass program (shape-independent).

    reps > 1 wraps the computation in a device-side For_i loop (for timing:
    the axon RPC wall-clock floor is ~100ms, so K iterations on-device make
    the kernel time measurable as a slope)."""
    import concourse.bacc as bacc
    import concourse.mybir as mybir
    import concourse.tile as tile

    nc = bacc.Bacc(None)
    bf16 = mybir.dt.bfloat16
    f32 = mybir.dt.float32
    adt = getattr(mybir.dt, a_dtype)

    # x host-swizzled to tile layout: xs[p, t*128 + f] = x[t*128 + p, f]
    xs_p = nc.declare_dram_parameter("xs", [128, NTB * 128], bf16, isOutput=False)
    w_p = nc.declare_dram_parameter("W", [128, 128], bf16, isOutput=False)
    deg2d_p = nc.declare_dram_parameter("deg2d", [128, NTB], f32, isOutput=False)
    degw_p = nc.declare_dram_parameter("degw", [128, DSTPAD], f32, isOutput=False)
    bias_p = nc.declare_dram_parameter("bias", [128, 1], f32, isOutput=False)
    a_p = nc.declare_dram_parameter("A", [APAD, DSTPAD], adt, isOutput=False)
    out_p = nc.declare_dram_parameter("out", [128, DSTPAD], f32, isOutput=True)

    with tile.TileContext(nc) as tc:
        with (
            tc.tile_pool(name="persist", bufs=1) as pp,
            tc.tile_pool(name="xdp", bufs=6) as xdp,
            tc.tile_pool(name="aps", bufs=1, space="PSUM") as aps,
            tc.tile_pool(name="ops", bufs=1, space="PSUM") as ops,
            tc.tile_pool(name="ap_sb", bufs=4) as ap_sb,
        ):
            xs = pp.tile([128, NTB * 128], bf16)
            XCH = 4  # load x in 4 slices so scaling can start early
            for i in range(XCH):
                sl = slice(i * NTB * 128 // XCH, (i + 1) * NTB * 128 // XCH)
                nc.sync.dma_start(xs[:, sl], xs_p[:, sl])
            w_sb = pp.tile([128, 128], bf16)
            nc.sync.dma_start(w_sb[:], w_p[:])
            deg2d = pp.tile([128, NTB], f32)
            nc.sync.dma_start(deg2d[:], deg2d_p[:])
            degw = pp.tile([128, DSTPAD], f32)
            nc.sync.dma_start(degw[:], degw_p[:])
            bias_sb = pp.tile([128, 1], f32)
            nc.sync.dma_start(bias_sb[:], bias_p[:])

            # dinv = 1/sqrt(deg): reciprocal on DVE, sqrt on ACT
            # (the Rsqrt activation is banned for accuracy reasons).
            dinv2d = pp.tile([128, NTB], f32)
            nc.vector.reciprocal(dinv2d[:], deg2d[:])
            nc.scalar.sqrt(dinv2d[:], dinv2d[:])
            dinvw = pp.tile([128, DSTPAD], f32)
            nc.vector.reciprocal(dinvw[:], degw[:])
            nc.scalar.sqrt(dinvw[:], dinvw[:])

            xasb = pp.tile([128, DSTPAD], bf16)
            outsb = pp.tile([128, DSTPAD], f32)
            if reps == 1:
                _emit_body(nc, mybir, adt, xs, w_sb, dinv2d, dinvw, bias_sb,
                           xasb, outsb, a_p, out_p, xdp, aps, ops, ap_sb)
            else:
                # hint_engines arms the branch prefetcher so the back-edge
                # IRAM refetch (~4us for >256-inst bodies) doesn't pollute
                # the per-iteration timing measurement
                hints = (mybir.EngineType.PE, mybir.EngineType.SP,
                         mybir.EngineType.DVE, mybir.EngineType.Activation)
                with tc.For_i(0, reps, 1, hint_engines=hints):
                    _emit_body(nc, mybir, adt, xs, w_sb, dinv2d, dinvw, bias_sb,
                               xasb, outsb, a_p, out_p, xdp, aps, ops, ap_sb)

    nc.finalize()
    return nc


def _emit_body(nc, mybir, adt, xs, w_sb, dinv2d, dinvw, bias_sb, xasb, outsb,
               a_p, out_p, xdp, aps, ops, ap_sb):
    bf16 = mybir.dt.bfloat16
    f32 = mybir.dt.float32
    # xa[f_in, dst] accumulator chunks (3 PSUM banks) and final out chunks
    xa = []
    po = []
    for c in range(NCH):
        w = min(PCH, DSTPAD - c * PCH)
        xac = aps.tile([128, w], f32, tag=f"xa{c}")
        xa.append(xac)
        poc = ops.tile([128, w], f32, tag=f"po{c}")
        po.append(poc)

    # ---- contract xd_t^T @ A_t over source tiles ------------------
    for g in range(NG):
        glen = min(TPD, NTB - g * TPD)
        at = ap_sb.tile([128, TPD, DSTPAD], adt, tag="at")
        nc.sync.dma_start(
            at[:, :glen, :],
            a_p[g * TPD * 128 : (g * TPD + glen) * 128, :].rearrange(
                "(g p) d -> p g d", p=128
            ),
        )
        for gg in range(glen):
            t = g * TPD + gg
            # xd_t = dinv_t * x_t (bf16), alternating ACT/DVE
            xd = xdp.tile([128, 128], bf16, tag="xd")
            if t % 2 == 0:
                nc.scalar.activation(
                    xd[:],
                    xs[:, t * 128 : (t + 1) * 128],
                    mybir.ActivationFunctionType.Copy,
                    scale=dinv2d[:, t : t + 1],
                )
            else:
                nc.vector.tensor_scalar_mul(
                    xd[:], xs[:, t * 128 : (t + 1) * 128], dinv2d[:, t : t + 1]
                )
            for c in range(NCH):
                w0 = c * PCH
                w1 = min(w0 + PCH, DSTPAD)
                nc.tensor.matmul(
                    out=xa[c][:],
                    lhsT=xd[:],
                    rhs=at[:, gg, w0:w1],
                    start=(t == 0),
                    stop=(t == NTB - 1),
                )
    # ---- tail: xa -> SBUF, apply W, postscale + bias + store ------
    # pipelined per chunk: copy (ACT/DVE) -> W matmul (PE) -> *dinv_dst
    # (DVE) -> +bias (ACT) -> chunk store, so chunk c+1 overlaps chunk c
    for c in range(NCH):
        w0 = c * PCH
        w1 = min(w0 + PCH, DSTPAD)
        if c % 2 == 0:
            nc.scalar.copy(xasb[:, w0:w1], xa[c][:])
        else:
            nc.vector.tensor_copy(xasb[:, w0:w1], xa[c][:])
        nc.tensor.matmul(
            out=po[c][:],
            lhsT=w_sb[:],
            rhs=xasb[:, w0:w1],
            start=True,
            stop=True,
        )
        nc.vector.tensor_tensor(
            out=outsb[:, w0:w1],
            in0=po[c][:],
            in1=dinvw[:, w0:w1],
            op=mybir.AluOpType.mult,
        )
        nc.scalar.add(outsb[:, w0:w1], outsb[:, w0:w1], bias_sb[:, 0:1])
        nc.sync.dma_start(out_p[:, w0:w1], outsb[:, w0:w1])


def _prep_inputs(x, adj, W, b, a_dtype="float8e4"):
    """Host-side sharding/layout: per-core dense count matrix, casts,
    transposes/swizzles. No numeric computation happens here (degrees are
    counts; rsqrt/scaling/matmuls run on-device)."""
    bf = ml_dtypes.bfloat16
    src = np.asarray(adj[0], dtype=np.int64)
    dst = np.asarray(adj[1], dtype=np.int64)
    x = np.asarray(x, dtype=np.float32)
    W = np.asarray(W, dtype=np.float32)
    b = np.asarray(b, dtype=np.float32)
    n = x.shape[0]
    assert n == N_NODES and x.shape[1] == D

    # self-loops as ordinary edges
    loops = np.arange(n, dtype=np.int64)
    allsrc = np.concatenate([src, loops])
    alldst = np.concatenate([dst, loops])

    deg = np.bincount(alldst, minlength=n).astype(np.float32)  # includes loops
    deg_pad = np.ones(NPAD, dtype=np.float32)
    deg_pad[:n] = deg

    xpad = np.zeros((NPAD, D), dtype=np.float32)
    xpad[:n] = x
    # swizzle to tile layout: xs[p, t*128 + f] = x[t*128 + p, f]
    xs = np.ascontiguousarray(
        xpad.reshape(NTB, 128, D).transpose(1, 0, 2).reshape(128, NTB * D)
    ).astype(bf)
    W16 = W.astype(bf)
    deg2d = np.ascontiguousarray(deg_pad.reshape(NTB, 128).T)
    bias = np.ascontiguousarray(b.reshape(D, 1))

    corea = alldst // PER_CORE
    loc = alldst - corea * PER_CORE
    in_maps = []
    for c in range(N_CORES):
        m = corea == c
        key = allsrc[m] * DSTPAD + loc[m]
        counts = np.bincount(key, minlength=APAD * DSTPAD)
        adt = np.dtype("float8_e4m3") if a_dtype == "float8e4" else bf
        A = counts.reshape(APAD, DSTPAD).astype(adt)
        degw = np.tile(deg_pad[c * PER_CORE : c * PER_CORE + DSTPAD][None, :], (128, 1))
        in_maps.append(
            {
                "xs": xs,
                "W": W16,
                "deg2d": deg2d,
                "degw": np.ascontiguousarray(degw),
                "bias": bias,
                "A": A,
            }
        )
    return in_maps


def kernel(x, adj, W, b):
    from concourse.bass_utils import run_bass_kernel_spmd

    # edge multiplicities up to 16 are exact in fp8e4; else use bf16
    dst = np.asarray(adj[1], dtype=np.int64)
    src = np.asarray(adj[0], dtype=np.int64)
    maxmult = int(np.bincount(src * np.int64(N_NODES) + dst).max())
    a_dtype = "float8e4" if maxmult + 1 <= 16 else "bfloat16"
    if a_dtype not in _cache:
        _cache[a_dtype] = _build_program(a_dtype=a_dtype)
    nc = _cache[a_dtype]
    in_maps = _prep_inputs(x, adj, W, b, a_dtype)
    res = run_bass_kernel_spmd(nc, in_maps, list(range(N_CORES)))
    out = np.empty((N_NODES, D), dtype=np.float32)
    for c in range(N_CORES):
        ot = res.results[c]["out"]  # [128, 1250] = out^T
        out[c * PER_CORE : (c + 1) * PER_CORE] = ot.T[:PER_CORE]
    return out


# revision 14
# speedup vs baseline: 1.6155x; 1.1008x over previous
"""GCN layer (PyG GCNConv semantics) on 8 Trainium2 NeuronCores.

out = D^{-1/2} (A + I) D^{-1/2} (x @ W) + b

Strategy (graph/data parallel, destinations sharded across cores):
  - Factor the symmetric norm and REASSOCIATE the contraction:
        out^T = W^T @ ((dinv * x)^T @ (A+I)) * dinv_dst + b
    i.e. contract the scaled features with the adjacency FIRST
    (xa[f_in, dst] = sum_s dinv_s x[s, f_in] A[s, dst]), then apply the
    128x128 weight once at the end. This removes the per-tile x@W
    matmuls entirely: TensorE does one pass over A per source tile plus
    a single trailing W application.
  - Each core owns a 1250-destination slice. The host re-encodes its
    edge bucket as a dense count matrix A_c [10240 src, 1250 dst]
    (fp8e4: counts <=16 are exact; self-loops included) —
    a pure structural re-encoding, streamed tile-by-tile at line rate.
  - The scaled features are split into an fp8 value + fp8 residual pair
    (xd8 = fp8(dinv*x), xr8 = fp8(dinv*x - xd8); their sum carries
    ~bf16 precision), enabling fp8 DoubleRow matmuls that contract TWO
    source tiles per instruction at 2 A-columns/cycle — halving
    TensorE streaming time vs a bf16xfp8 stream.
  - Tail: xa -> SBUF (bf16), out^T = W^T @ xa, postscale by dinv[dst],
    add bias, store out^T per chunk; host reassembles.
"""

import sys

for _p in ("/opt/trn_rl_repo", "/root/.axon_site/_ro/trn_rl_repo"):
    if _p not in sys.path:
        sys.path.append(_p)

import numpy as np
import ml_dtypes

N_NODES = 10000
N_CORES = 8
PER_CORE = 1250  # dst nodes per core
D = 128
NTB = 80  # source tiles (node padding to 10240; tile 79 is all zero)
NPAD = NTB * 128  # 10240
APAD = NPAD
NPAIR = NTB // 2  # 40 DoubleRow tile pairs
DSTPAD = 1250  # per-core dst count
TPD = 8  # A tiles per DMA group (4 pairs)
NG = NTB // TPD  # 10 groups
PCH = 512  # psum chunk for the W application
NCH = (DSTPAD + PCH - 1) // PCH  # 3 chunks: 512, 512, 226
# DoubleRow moving free dim is 2*chunk <= 512 -> dst chunks of <=256
CHUNKS5 = [(0, 256), (256, 256), (512, 256), (768, 256), (1024, 226)]
LAG = 2  # pairs of pipeline slack between quantization and contraction

_cache = {}


def _build_program(reps=1, a_dtype="float8e4"):
    """Build + finalize the SPMD Bass program (shape-independent).

    reps > 1 wraps the computation in a device-side For_i loop (for timing:
    the axon RPC wall-clock floor is ~100ms, so K iterations on-device make
    the kernel time measurable as a slope)."""
    import concourse.bacc as bacc
    import concourse.mybir as mybir
    import concourse.tile as tile

    nc = bacc.Bacc(None)
    bf16 = mybir.dt.bfloat16
    f32 = mybir.dt.float32
    f8 = mybir.dt.float8e4
    adt = getattr(mybir.dt, a_dtype)
    fp8_path = a_dtype == "float8e4"

    # x host-swizzled to tile layout: xs[p, t*128 + f] = x[t*128 + p, f]
    xs_p = nc.declare_dram_parameter("xs", [128, NTB * 128], bf16, isOutput=False)
    w_p = nc.declare_dram_parameter("W", [128, 128], bf16, isOutput=False)
    deg2d_p = nc.declare_dram_parameter("deg2d", [128, NTB], f32, isOutput=False)
    degw_p = nc.declare_dram_parameter("degw", [128, DSTPAD], f32, isOutput=False)
    bias_p = nc.declare_dram_parameter("bias", [128, 1], f32, isOutput=False)
    a_p = nc.declare_dram_parameter("A", [APAD, DSTPAD], adt, isOutput=False)
    out_p = nc.declare_dram_parameter("out", [128, DSTPAD], f32, isOutput=True)

    with tile.TileContext(nc) as tc:
        with (
            tc.tile_pool(name="persist", bufs=1) as pp,
            tc.tile_pool(name="xdp", bufs=6) as xdp,
            tc.tile_pool(name="aps", bufs=1, space="PSUM") as aps,
            tc.tile_pool(name="ops", bufs=1, space="PSUM") as ops,
            tc.tile_pool(name="ap_sb", bufs=6) as ap_sb,
        ):
            xs = pp.tile([128, NTB * 128], bf16)
            XCH = 4  # load x in 4 slices so scaling can start early
            for i in range(XCH):
                sl = slice(i * NTB * 128 // XCH, (i + 1) * NTB * 128 // XCH)
                nc.sync.dma_start(xs[:, sl], xs_p[:, sl])
            w_sb = pp.tile([128, 128], bf16)
            nc.sync.dma_start(w_sb[:], w_p[:])
            deg2d = pp.tile([128, NTB], f32)
            nc.sync.dma_start(deg2d[:], deg2d_p[:])
            degw = pp.tile([128, DSTPAD], f32)
            nc.sync.dma_start(degw[:], degw_p[:])
            bias_sb = pp.tile([128, 1], f32)
            nc.sync.dma_start(bias_sb[:], bias_p[:])

            # dinv = 1/sqrt(deg): reciprocal on DVE, sqrt on ACT
            # (the Rsqrt activation is banned for accuracy reasons).
            dinv2d = pp.tile([128, NTB], f32)
            nc.vector.reciprocal(dinv2d[:], deg2d[:])
            nc.scalar.sqrt(dinv2d[:], dinv2d[:])
            dinvw = pp.tile([128, DSTPAD], f32)
            nc.vector.reciprocal(dinvw[:], degw[:])
            nc.scalar.sqrt(dinvw[:], dinvw[:])

            xasb = pp.tile([128, DSTPAD], bf16)
            outsb = pp.tile([128, DSTPAD], f32)
            if fp8_path:
                xd8 = pp.tile([128, NPAIR, 2, 128], f8)
                xr8 = pp.tile([128, NPAIR, 2, 128], f8)
                # tile 79 is pure padding: zero once so the contraction
                # never multiplies uninitialized fp8 (NaN) into PSUM
                nc.vector.memset(xd8[:, NPAIR - 1, 1, :], 0.0)
                nc.vector.memset(xr8[:, NPAIR - 1, 1, :], 0.0)
                state = (xd8, xr8)
                emit = _emit_body_fp8
            else:
                state = ()
                emit = _emit_body_bf16

            args = (nc, mybir, adt, xs, w_sb, dinv2d, dinvw, bias_sb, state,
                    xasb, outsb, a_p, out_p, xdp, aps, ops, ap_sb)
            if reps == 1:
                emit(*args)
            else:
                # hint_engines arms the branch prefetcher so the back-edge
                # IRAM refetch (~4us for >256-inst bodies) doesn't pollute
                # the per-iteration timing measurement
                hints = (mybir.EngineType.PE, mybir.EngineType.SP,
                         mybir.EngineType.DVE, mybir.EngineType.Activation)
                with tc.For_i(0, reps, 1, hint_engines=hints):
                    emit(*args)

    nc.finalize()
    return nc


def _emit_tail(nc, mybir, w_sb, dinvw, bias_sb, xa, xasb, outsb, out_p, ops,
               xa_chunks):
    """xa (PSUM chunks) -> SBUF bf16 -> W apply -> postscale/bias/store,
    pipelined per 512-chunk."""
    f32 = mybir.dt.float32
    po = []
    for c in range(NCH):
        w = min(PCH, DSTPAD - c * PCH)
        poc = ops.tile([128, w], f32, tag=f"po{c}")
        po.append(poc)
    # copy xa chunks to SBUF (alternating engines)
    for i, (c0, w) in enumerate(xa_chunks):
        if i % 2 == 0:
            nc.scalar.copy(xasb[:, c0 : c0 + w], xa[i][:])
        else:
            nc.vector.tensor_copy(xasb[:, c0 : c0 + w], xa[i][:])
    for c in range(NCH):
        w0 = c * PCH
        w1 = min(w0 + PCH, DSTPAD)
        nc.tensor.matmul(
            out=po[c][:],
            lhsT=w_sb[:],
            rhs=xasb[:, w0:w1],
            start=True,
            stop=True,
        )
        nc.vector.tensor_tensor(
            out=outsb[:, w0:w1],
            in0=po[c][:],
            in1=dinvw[:, w0:w1],
            op=mybir.AluOpType.mult,
        )
        nc.scalar.add(outsb[:, w0:w1], outsb[:, w0:w1], bias_sb[:, 0:1])
        nc.sync.dma_start(out_p[:, w0:w1], outsb[:, w0:w1])


def _emit_body_fp8(nc, mybir, adt, xs, w_sb, dinv2d, dinvw, bias_sb, state,
                   xasb, outsb, a_p, out_p, xdp, aps, ops, ap_sb):
    xd8, xr8 = state
    f32 = mybir.dt.float32
    DR = mybir.MatmulPerfMode.DoubleRow
    xa = []
    for c, (w0, w) in enumerate(CHUNKS5):
        xac = aps.tile([128, w], f32, tag=f"xa{c}")
        xa.append(xac)
    at_tiles = {}

    def load_group(g):
        at = ap_sb.tile([128, TPD, DSTPAD], adt, tag="at")
        # split the group DMA across two issuing engines -> two HW-DGE
        # queue sets run in parallel
        half = TPD // 2
        nc.sync.dma_start(
            at[:, :half, :],
            a_p[g * TPD * 128 : (g * TPD + half) * 128, :].rearrange(
                "(g p) d -> p g d", p=128
            ),
        )
        nc.scalar.dma_start(
            at[:, half:, :],
            a_p[(g * TPD + half) * 128 : (g + 1) * TPD * 128, :].rearrange(
                "(g p) d -> p g d", p=128
            ),
        )
        at_tiles[g] = at

    def quant_tile(t):
        # xd8 = fp8(dinv*x) on ACT; xr8 = fp8(dinv*x - xd8) fused on DVE
        p, i = t // 2, t % 2
        xt = xs[:, t * 128 : (t + 1) * 128]
        nc.scalar.activation(
            xd8[:, p, i, :],
            xt,
            mybir.ActivationFunctionType.Copy,
            scale=dinv2d[:, t : t + 1],
        )
        nc.vector.scalar_tensor_tensor(
            xr8[:, p, i, :],
            xt,
            dinv2d[:, t : t + 1],
            xd8[:, p, i, :],
            op0=mybir.AluOpType.mult,
            op1=mybir.AluOpType.subtract,
        )

    def contract_pair(p):
        # pass-outer: the 5 dst-chunk matmuls of one pass share lhsT (the
        # PE skips re-loading unchanged stationary weights)
        g, gi = p // 4, (p % 4) * 2
        at = at_tiles[g]
        for k, hh in enumerate((xd8, xr8)):
            for c, (w0, w) in enumerate(CHUNKS5):
                nc.tensor.matmul(
                    out=xa[c][:],
                    lhsT=hh[:, p, :, :],
                    rhs=at[:, gi : gi + 2, w0 : w0 + w],
                    start=(p == 0 and k == 0),
                    stop=(p == NPAIR - 1 and k == 1),
                    perf_mode=DR,
                )

    load_group(0)
    load_group(1)
    for p in range(NPAIR):
        for i in (0, 1):
            t = 2 * p + i
            if t < NTB - 1:
                quant_tile(t)
        if p % 4 == 0 and p // 4 + 2 < NG:
            load_group(p // 4 + 2)
        if p >= LAG:
            contract_pair(p - LAG)
    for p in range(NPAIR - LAG, NPAIR):
        contract_pair(p)

    _emit_tail(nc, mybir, w_sb, dinvw, bias_sb, xa, xasb, outsb, out_p, ops,
               CHUNKS5)


def _emit_body_bf16(nc, mybir, adt, xs, w_sb, dinv2d, dinvw, bias_sb, state,
                    xasb, outsb, a_p, out_p, xdp, aps, ops, ap_sb):
    """Fallback for pathological inputs (edge multiplicity > 15): bf16 A
    stream with bf16 scaled features, 1 A-column/cycle."""
    bf16 = mybir.dt.bfloat16
    f32 = mybir.dt.float32
    xa = []
    xa_chunks = []
    for c in range(NCH):
        w0 = c * PCH
        w = min(PCH, DSTPAD - w0)
        xac = aps.tile([128, w], f32, tag=f"xa{c}")
        xa.append(xac)
        xa_chunks.append((w0, w))
    for g in range(NG):
        glen = min(TPD, NTB - g * TPD)
        at = ap_sb.tile([128, TPD, DSTPAD], adt, tag="at")
        nc.sync.dma_start(
            at[:, :glen, :],
            a_p[g * TPD * 128 : (g * TPD + glen) * 128, :].rearrange(
                "(g p) d -> p g d", p=128
            ),
        )
        for gg in range(glen):
            t = g * TPD + gg
            xd = xdp.tile([128, 128], bf16, tag="xd")
            if t % 2 == 0:
                nc.scalar.activation(
                    xd[:],
                    xs[:, t * 128 : (t + 1) * 128],
                    mybir.ActivationFunctionType.Copy,
                    scale=dinv2d[:, t : t + 1],
                )
            else:
                nc.vector.tensor_scalar_mul(
                    xd[:], xs[:, t * 128 : (t + 1) * 128], dinv2d[:, t : t + 1]
                )
            for c in range(NCH):
                w0 = c * PCH
                w1 = min(w0 + PCH, DSTPAD)
                nc.tensor.matmul(
                    out=xa[c][:],
                    lhsT=xd[:],
                    rhs=at[:, gg, w0:w1],
                    start=(t == 0),
                    stop=(t == NTB - 1),
                )
    _emit_tail(nc, mybir, w_sb, dinvw, bias_sb, xa, xasb, outsb, out_p, ops,
               xa_chunks)


def _prep_inputs(x, adj, W, b, a_dtype="float8e4"):
    """Host-side sharding/layout: per-core dense count matrix, casts,
    transposes/swizzles. No numeric computation happens here (degrees are
    counts; rsqrt/scaling/matmuls run on-device)."""
    bf = ml_dtypes.bfloat16
    src = np.asarray(adj[0], dtype=np.int64)
    dst = np.asarray(adj[1], dtype=np.int64)
    x = np.asarray(x, dtype=np.float32)
    W = np.asarray(W, dtype=np.float32)
    b = np.asarray(b, dtype=np.float32)
    n = x.shape[0]
    assert n == N_NODES and x.shape[1] == D

    # self-loops as ordinary edges
    loops = np.arange(n, dtype=np.int64)
    allsrc = np.concatenate([src, loops])
    alldst = np.concatenate([dst, loops])

    deg = np.bincount(alldst, minlength=n).astype(np.float32)  # includes loops
    deg_pad = np.ones(NPAD, dtype=np.float32)
    deg_pad[:n] = deg

    xpad = np.zeros((NPAD, D), dtype=np.float32)
    xpad[:n] = x
    # swizzle to tile layout: xs[p, t*128 + f] = x[t*128 + p, f]
    xs = np.ascontiguousarray(
        xpad.reshape(NTB, 128, D).transpose(1, 0, 2).reshape(128, NTB * D)
    ).astype(bf)
    W16 = W.astype(bf)
    deg2d = np.ascontiguousarray(deg_pad.reshape(NTB, 128).T)
    bias = np.ascontiguousarray(b.reshape(D, 1))

    corea = alldst // PER_CORE
    loc = alldst - corea * PER_CORE
    in_maps = []
    for c in range(N_CORES):
        m = corea == c
        key = allsrc[m] * DSTPAD + loc[m]
        counts = np.bincount(key, minlength=APAD * DSTPAD)
        adt = np.dtype("float8_e4m3") if a_dtype == "float8e4" else bf
        A = counts.reshape(APAD, DSTPAD).astype(adt)
        degw = np.tile(deg_pad[c * PER_CORE : c * PER_CORE + DSTPAD][None, :], (128, 1))
        in_maps.append(
            {
                "xs": xs,
                "W": W16,
                "deg2d": deg2d,
                "degw": np.ascontiguousarray(degw),
                "bias": bias,
                "A": A,
            }
        )
    return in_maps


def kernel(x, adj, W, b):
    from concourse.bass_utils import run_bass_kernel_spmd

    # edge multiplicities up to 16 are exact in fp8e4; else use bf16
    dst = np.asarray(adj[1], dtype=np.int64)
    src = np.asarray(adj[0], dtype=np.int64)
    maxmult = int(np.bincount(src * np.int64(N_NODES) + dst).max())
    a_dtype = "float8e4" if maxmult + 1 <= 16 else "bfloat16"
    if a_dtype not in _cache:
        _cache[a_dtype] = _build_program(a_dtype=a_dtype)
    nc = _cache[a_dtype]
    in_maps = _prep_inputs(x, adj, W, b, a_dtype)
    res = run_bass_kernel_spmd(nc, in_maps, list(range(N_CORES)))
    out = np.empty((N_NODES, D), dtype=np.float32)
    for c in range(N_CORES):
        ot = res.results[c]["out"]  # [128, 1250] = out^T
        out[c * PER_CORE : (c + 1) * PER_CORE] = ot.T[:PER_CORE]
    return out
